# revision 32
# baseline (speedup 1.0000x reference)
"""Trainium2 Bass kernel for nn_MultiHeadAttention_85229331022244.

Computation (per batch b):
  xh = x.reshape(B,T,64,16); q/k/v = per-head 64x64 projections of xh
  q,k: interleaved RoPE over the FULL 1024-dim feature axis
  scores = q @ k.T / sqrt(1024)  (single attention map over full D)
  causal softmax; y = attn @ v

Sharding: core b -> batch b (4 cores used).  One core owns the whole
batch, so x is shipped to the device exactly once and K/Q/V projections
share the same x tiles.  Host<->device traffic is the metric driver
(memory regime): RoPE cos/sin tables are reconstructed on-device from
factorized half-tables (angle addition), causal masks are generated
on-device with affine_select, and the output returns as bf16.

Device layout trick: heads are reordered even-first and paired so the
projections become 8 block-diagonal 128x128 matmuls that produce
K^T/Q^T directly in [feature-on-partition, token] layout, with RoPE
partner features living in chunk c and c+4 at the same partition index.
"""

import math
from contextlib import ExitStack

import numpy as np
import ml_dtypes

import concourse.bass as bass
import concourse.mybir as mybir
import concourse.tile as tile
from concourse import bacc
from concourse.bass import ts, ds
from concourse.masks import make_identity

BF16 = ml_dtypes.bfloat16

D_MODEL = 1024
N_HEADS = 16
HEAD_D = 64
ROPE_BASE = 10000.0
GAMMA = 1.0 / math.sqrt(D_MODEL)
T = 4096
NT = 32  # 128-row query tiles per batch
NS = 8   # 512-token stripes

# head pairs per 128-row chunk; chunks 0-3 = even heads, 4-7 = odd heads
HEAD_PAIRS = [(0, 2), (4, 6), (8, 10), (12, 14), (1, 3), (5, 7), (9, 11), (13, 15)]


def _feature_perm():
    """perm[c*128 + p] = original feature index for kernel row (c, p)."""
    perm = np.zeros(1024, dtype=np.int64)
    for c, (ha, hb) in enumerate(HEAD_PAIRS):
        for p in range(128):
            h = ha if p < 64 else hb
            perm[c * 128 + p] = (p % 64) * 16 + h
    return perm


PERM = _feature_perm()
INV_PERM = np.argsort(PERM)


def _block_weights(w):
    """w: (64, 64, 16) -> (8, 128, 128) block-diag per chunk, bf16."""
    out = np.zeros((8, 128, 128), dtype=np.float32)
    for c, (ha, hb) in enumerate(HEAD_PAIRS):
        out[c, :64, :64] = w[:, :, ha]
        out[c, 64:, 64:] = w[:, :, hb]
    return out.astype(BF16)


def _inv_freq():
    """[4, 128] rope inverse frequencies for chunks 0-3 (partners 4-7)."""
    p = np.arange(128)
    out = np.zeros((4, 128), dtype=np.float64)
    for c in range(4):
        f = (p % 64) * 8 + (2 * c + p // 64)
        out[c] = ROPE_BASE ** (-f.astype(np.float64) / 512.0)
    return out


def _rope_factor_tables():
    """Two-level angle factorization: ang(p, 512*s + 64*m + v)
    = hi(p,s) + mid(p,m) + low(p,v).  All fp32 (device rebuilds cosL/sinL
    in fp32 then rounds to bf16 once, matching single-level precision).

    Returns cosH/sinH [4,128,NS], cosM/sinM [4,128,8], cosV/sinV [4,128,64].
    """
    invf = _inv_freq()  # [4, 128]
    v = np.arange(64, dtype=np.float64)
    m = np.arange(8, dtype=np.float64) * 64.0
    s = np.arange(NS, dtype=np.float64) * 512.0
    low = invf[:, :, None] * v[None, None, :]   # [4,128,64]
    mid = invf[:, :, None] * m[None, None, :]   # [4,128,8]
    hi = invf[:, :, None] * s[None, None, :]    # [4,128,NS]
    f32 = np.float32
    return (
        np.cos(hi).astype(f32), np.sin(hi).astype(f32),
        np.cos(mid).astype(f32), np.sin(mid).astype(f32),
        np.cos(low).astype(f32), np.sin(low).astype(f32),
    )


def build_nc():
    """Build the (identical-on-all-cores) Bass program for one full batch."""
    dt = mybir.dt
    nc = bacc.Bacc("TRN2", target_bir_lowering=False)
    xpT = nc.dram_tensor("xpT", [8, 128, T], dt.bfloat16, kind="ExternalInput")
    # compact block-diag weights (only the 64 nonzero cols per row):
    # wq|wk|wv, 8 chunks x 64 cols each -> expanded on device
    wtab = nc.dram_tensor("wtab", [128, 1536], dt.bfloat16, kind="ExternalInput")
    # fp32 rope factor tables: cosH|sinH (4x8 each), cosM|sinM (4x8 each),
    # cosV|sinV (4x64 each); cosL/sinL are reconstructed on device
    httab = nc.dram_tensor("httab", [128, 640], dt.float32, kind="ExternalInput")
    y = nc.dram_tensor("y", [T, 1024], dt.bfloat16, kind="ExternalOutput")

    with tile.TileContext(nc) as tc, ExitStack() as ctx:
        const = ctx.enter_context(tc.tile_pool(name="const", bufs=1))
        kv = ctx.enter_context(tc.tile_pool(name="kv", bufs=1))
        qpool = ctx.enter_context(tc.tile_pool(name="qpool", bufs=2))
        xpool = ctx.enter_context(tc.tile_pool(name="xpool", bufs=2))
        cspool = ctx.enter_context(tc.tile_pool(name="cspool", bufs=2))
        rtmp = ctx.enter_context(tc.tile_pool(name="rtmp", bufs=2))
        ppool = ctx.enter_context(tc.tile_pool(name="ppool", bufs=2))
        ptpool = ctx.enter_context(tc.tile_pool(name="ptpool", bufs=2))
        ypool = ctx.enter_context(tc.tile_pool(name="ypool", bufs=2))
        lpool = ctx.enter_context(tc.tile_pool(name="lpool", bufs=2))
        psum = ctx.enter_context(tc.tile_pool(name="psum", bufs=2, space="PSUM"))
        # YL/YH double-buffered; V-projection PSUM shares the same slots
        # (proj and attention never need them simultaneously beyond the
        # rotation the scheduler already enforces).
        psum1 = ctx.enter_context(tc.tile_pool(name="psum1", bufs=2, space="PSUM"))

        # constants
        ident = const.tile([128, 128], dt.bfloat16, tag="ident", name="ident")
        make_identity(nc, ident)
        wtab_sb = const.tile([128, 1536], dt.bfloat16, tag="wtab", name="wtab")
        nc.sync.dma_start(wtab_sb[:], wtab[:])
        httab_sb = const.tile([128, 640], dt.float32, tag="httab", name="httab")
        nc.sync.dma_start(httab_sb[:], httab[:])
        # expand compact weights to 128x128 block-diagonal tiles
        wq_sb, wk_sb, wv_sb = [], [], []
        for wi, lst in ((0, wq_sb), (1, wk_sb), (2, wv_sb)):
            for c in range(8):
                wt = const.tile([128, 128], dt.bfloat16, tag=f"w{wi}_{c}",
                                name=f"w{wi}_{c}")
                nc.gpsimd.memset(wt[:], 0.0)
                off = wi * 512 + c * 64
                nc.gpsimd.tensor_copy(wt[0:64, 0:64], wtab_sb[0:64, ds(off, 64)])
                nc.gpsimd.tensor_copy(wt[64:128, 64:128],
                                      wtab_sb[64:128, ds(off, 64)])
                lst.append(wt)
        # reconstruct the rope "low" tables: cos/sin(invf * u), u = 64*m + v
        cosL_sb, sinL_sb = [], []
        for c in range(4):
            cv = httab_sb[:, ds(128 + c * 64, 64)]
            sv = httab_sb[:, ds(384 + c * 64, 64)]
            clt = const.tile([128, 512], dt.bfloat16, tag=f"cl{c}", name=f"cl{c}")
            slt = const.tile([128, 512], dt.bfloat16, tag=f"sl{c}", name=f"sl{c}")
            for m in range(NS):
                cm = httab_sb[:, ds(64 + c * NS + m, 1)]
                sm = httab_sb[:, ds(96 + c * NS + m, 1)]
                u1 = cspool.tile([128, 64], dt.float32, tag="u1", name="u1")
                u2 = cspool.tile([128, 64], dt.float32, tag="u2", name="u2")
                nc.gpsimd.tensor_scalar_mul(u1[:], cv, cm)
                nc.gpsimd.tensor_scalar_mul(u2[:], sv, sm)
                nc.gpsimd.tensor_sub(clt[:, ds(m * 64, 64)], u1[:], u2[:])
                u3 = cspool.tile([128, 64], dt.float32, tag="u1", name="u3")
                u4 = cspool.tile([128, 64], dt.float32, tag="u2", name="u4")
                nc.gpsimd.tensor_scalar_mul(u3[:], cv, sm)
                nc.gpsimd.tensor_scalar_mul(u4[:], sv, cm)
                nc.gpsimd.tensor_add(slt[:, ds(m * 64, 64)], u3[:], u4[:])
            cosL_sb.append(clt)
            sinL_sb.append(slt)
        cosH_sb = [httab_sb[:, ds(c * NS, NS)] for c in range(4)]
        sinH_sb = [httab_sb[:, ds(32 + c * NS, NS)] for c in range(4)]
        # causal masks for the last stripe of each q-tile: pattern depends only
        # on r = G mod 4.  mask_r[p, c] = 0 if c <= 128*r + p else -1e9.
        masks = []
        for r in range(4):
            mt = const.tile([128, 512], dt.float32, tag=f"mask{r}", name=f"mask{r}")
            nc.gpsimd.memset(mt[:], 0.0)
            nc.gpsimd.affine_select(
                out=mt[:],
                in_=mt[:],
                compare_op=mybir.AluOpType.is_ge,
                fill=-1e9,
                base=r * 128,
                pattern=[[-1, 512]],
                channel_multiplier=1,
            )
            masks.append(mt)

        # resident K^T (per chunk c and 512-key stripe s) and V (per-stripe)
        KT = {}
        for s in range(NS):
            for c in range(8):
                KT[(c, s)] = kv.tile([128, 512], dt.bfloat16, tag=f"kt{c}_{s}",
                                     name=f"kt{c}_{s}")
        V = [
            kv.tile([128, 4, 1024], dt.bfloat16, tag=f"v{s}", name=f"v{s}")
            for s in range(NS)
        ]
        QT = {}  # streamed, tags per chunk

        def emit_proj_stripe(s):
            """K^T, Q^T, V for the 512-token stripe s (shared x / cos / sin)."""
            sl = ds(s * 512, 512)
            for cp in range(4):
                xa = xpool.tile([128, 512], dt.bfloat16, tag="xa", name="xa")
                xb = xpool.tile([128, 512], dt.bfloat16, tag="xb", name="xb")
                nc.sync.dma_start(xa[:], xpT[cp, :, sl])
                nc.sync.dma_start(xb[:], xpT[cp + 4, :, sl])
                # reconstruct cos/sin for (chunk cp, stripe s) via angle addition
                cos = cspool.tile([128, 512], dt.bfloat16, tag="cos", name="cos")
                sin = cspool.tile([128, 512], dt.bfloat16, tag="sin", name="sin")
                t1 = cspool.tile([128, 512], dt.bfloat16, tag="t1", name="t1")
                t2 = cspool.tile([128, 512], dt.bfloat16, tag="t2", name="t2")
                chs = httab_sb[:, ds(cp * NS + s, 1)]
                shs = httab_sb[:, ds(32 + cp * NS + s, 1)]
                nc.vector.tensor_scalar_mul(t1[:], cosL_sb[cp][:], chs)
                nc.vector.tensor_scalar_mul(t2[:], sinL_sb[cp][:], shs)
                nc.vector.tensor_sub(cos[:], t1[:], t2[:])
                t3 = cspool.tile([128, 512], dt.bfloat16, tag="t1", name="t3")
                t4 = cspool.tile([128, 512], dt.bfloat16, tag="t2", name="t4")
                nc.vector.tensor_scalar_mul(t3[:], cosL_sb[cp][:], shs)
                nc.vector.tensor_scalar_mul(t4[:], sinL_sb[cp][:], chs)
                nc.vector.tensor_add(sin[:], t3[:], t4[:])

                def rope_pair(w_sb, out_e, out_o):
                    pe = psum.tile([128, 512], dt.float32, tag="A", name="A")
                    po = psum.tile([128, 512], dt.float32, tag="B", name="B")
                    nc.tensor.matmul(pe[:], lhsT=w_sb[cp][:], rhs=xa[:],
                                     start=True, stop=True)
                    nc.tensor.matmul(po[:], lhsT=w_sb[cp + 4][:], rhs=xb[:],
                                     start=True, stop=True)
                    ke = rtmp.tile([128, 512], dt.bfloat16, tag="ke", name="ke")
                    ko = rtmp.tile([128, 512], dt.bfloat16, tag="ko", name="ko")
                    nc.scalar.copy(ke[:], pe[:])
                    nc.scalar.copy(ko[:], po[:])
                    ta = rtmp.tile([128, 512], dt.bfloat16, tag="ta", name="ta")
                    tb = rtmp.tile([128, 512], dt.bfloat16, tag="tb", name="tb")
                    nc.vector.tensor_mul(ta[:], ke[:], cos[:])
                    nc.vector.tensor_mul(tb[:], ko[:], sin[:])
                    nc.vector.tensor_sub(out_e[:], ta[:], tb[:])
                    ta2 = rtmp.tile([128, 512], dt.bfloat16, tag="ta", name="ta")
                    tb2 = rtmp.tile([128, 512], dt.bfloat16, tag="tb", name="tb")
                    nc.vector.tensor_mul(ta2[:], ke[:], sin[:])
                    nc.vector.tensor_mul(tb2[:], ko[:], cos[:])
                    nc.vector.tensor_add(out_o[:], ta2[:], tb2[:])

                rope_pair(wk_sb, KT[(cp, s)], KT[(cp + 4, s)])
                QT[(cp, s)] = qpool.tile([128, 512], dt.bfloat16, tag=f"qt{cp}",
                                         name=f"qt{cp}")
                QT[(cp + 4, s)] = qpool.tile([128, 512], dt.bfloat16,
                                             tag=f"qt{cp + 4}", name=f"qt{cp + 4}")
                rope_pair(wq_sb, QT[(cp, s)], QT[(cp + 4, s)])

                va = psum1.tile([128, 4, 128], dt.float32, tag="YL", name="VA")
                vb = psum1.tile([128, 4, 128], dt.float32, tag="YH", name="VB")
                for sub in range(4):
                    nc.tensor.matmul(
                        va[:, sub, :], lhsT=xa[:, ts(sub, 128)], rhs=wv_sb[cp][:],
                        start=True, stop=True,
                    )
                    nc.tensor.matmul(
                        vb[:, sub, :], lhsT=xb[:, ts(sub, 128)], rhs=wv_sb[cp + 4][:],
                        start=True, stop=True,
                    )
                nc.any.tensor_copy(V[s][:, :, ds(cp * 128, 128)], va[:])
                nc.any.tensor_copy(V[s][:, :, ds((cp + 4) * 128, 128)], vb[:])

        def emit_q_tile(G):
            nblk = G + 1
            nst = (nblk + 3) // 4
            wlast = (nblk - 4 * (nst - 1)) * 128
            qs, qoff = G // 4, (G % 4) * 128
            y_lo = psum1.tile([128, 512], dt.float32, tag="YL", name="YL")
            y_hi = psum1.tile([128, 512], dt.float32, tag="YH", name="YH")
            l_parts = lpool.tile([128, NS], dt.float32, tag="lp", name="lp")
            for t in range(nst):
                w = 512 if t < nst - 1 else wlast
                S = psum.tile([128, 512], dt.float32, tag="A", name="A")
                for c in range(8):
                    nc.tensor.matmul(
                        S[:, :w],
                        lhsT=QT[(c, qs)][:, ds(qoff, 128)],
                        rhs=KT[(c, t)][:, :w],
                        start=(c == 0), stop=(c == 7),
                    )
                if t == nst - 1:
                    nc.vector.tensor_add(S[:, :w], S[:, :w], masks[G % 4][:, :w])
                P = ppool.tile([128, 512], dt.bfloat16, tag="p", name="p")
                nc.scalar.activation(
                    P[:, :w], S[:, :w], mybir.ActivationFunctionType.Exp,
                    scale=GAMMA, accum_out=l_parts[:, ds(t, 1)],
                )
                nb = w // 128
                pt_ps = psum.tile([128, 512], dt.bfloat16, tag="B", name="B")
                for b in range(nb):
                    nc.tensor.transpose(pt_ps[:, ts(b, 128)], P[:, ts(b, 128)],
                                        ident[:])
                pt = ptpool.tile([128, 512], dt.bfloat16, tag="pt", name="pt")
                nc.scalar.copy(pt[:, :w], pt_ps[:, :w])
                for b in range(nb):
                    blk = t * 4 + b
                    vs = V[blk // 4]
                    nc.tensor.matmul(y_lo[:], lhsT=pt[:, ts(b, 128)],
                                     rhs=vs[:, blk % 4, 0:512],
                                     start=(blk == 0), stop=(blk == nblk - 1))
                    nc.tensor.matmul(y_hi[:], lhsT=pt[:, ts(b, 128)],
                                     rhs=vs[:, blk % 4, 512:1024],
                                     start=(blk == 0), stop=(blk == nblk - 1))
            lsum = lpool.tile([128, 1], dt.float32, tag="ls", name="ls")
            linv = lpool.tile([128, 1], dt.float32, tag="li", name="li")
            nc.vector.tensor_reduce(lsum[:], l_parts[:, :nst],
                                    mybir.AxisListType.X, mybir.AluOpType.add)
            nc.vector.reciprocal(linv[:], lsum[:])
            y_sb = ypool.tile([128, 1024], dt.bfloat16, tag="y", name="y")
            nc.vector.tensor_scalar_mul(y_sb[:, 0:512], y_lo[:], linv[:])
            nc.vector.tensor_scalar_mul(y_sb[:, 512:1024], y_hi[:], linv[:])
            nc.sync.dma_start(y[ts(G, 128), :], y_sb[:])

        # Projection runs one stripe ahead of attention so its DMA + DVE
        # latency hides under the (PE-bound) attention of the prior stripe.
        emit_proj_stripe(0)
        for s in range(NS):
            if s + 1 < NS:
                emit_proj_stripe(s + 1)
            for G in range(4 * s, 4 * s + 4):
                emit_q_tile(G)

    nc.compile()
    return nc


# ------------------------- host side -------------------------


def _compact_w(w2):
    """[8,128,128] block-diag -> [128, 8*64] nonzero cols per row."""
    out = np.zeros((128, 8 * 64), dtype=np.float32)
    w2 = np.asarray(w2, dtype=np.float32)
    for c in range(8):
        out[0:64, c * 64:(c + 1) * 64] = w2[c, 0:64, 0:64]
        out[64:128, c * 64:(c + 1) * 64] = w2[c, 64:128, 64:128]
    return out


def pack_tables(w2q, w2k, w2v, tables):
    """wtab [128,1536] bf16 and httab [128,640] fp32 (shared by all cores)."""
    cosH_t, sinH_t, cosM_t, sinM_t, cosV_t, sinV_t = tables
    wtab = np.concatenate(
        [_compact_w(w2q), _compact_w(w2k), _compact_w(w2v)], axis=1
    ).astype(BF16)
    httab = np.concatenate(
        [
            cosH_t.transpose(1, 0, 2).reshape(128, 32),
            sinH_t.transpose(1, 0, 2).reshape(128, 32),
            cosM_t.transpose(1, 0, 2).reshape(128, 32),
            sinM_t.transpose(1, 0, 2).reshape(128, 32),
            cosV_t.transpose(1, 0, 2).reshape(128, 256),
            sinV_t.transpose(1, 0, 2).reshape(128, 256),
        ],
        axis=1,
    ).astype(np.float32)
    return wtab, httab


def prep_core_inputs(xb, wtab, httab):
    """Inputs for one core: batch slice xb (T, 1024) fp32."""
    xpT = np.ascontiguousarray(xb.T[PERM].reshape(8, 128, T)).astype(BF16)
    return {"xpT": xpT, "wtab": wtab, "httab": httab}


def core_model(inp):
    """Numpy model of one core's program (fp32 math, for tests)."""
    xpT = inp["xpT"].astype(np.float32)
    wtab = inp["wtab"].astype(np.float32)
    httab = inp["httab"].astype(np.float32)

    def expand_w(block):  # [128, 512] compact -> [8,128,128] block-diag
        out = np.zeros((8, 128, 128), dtype=np.float32)
        for c in range(8):
            out[c, 0:64, 0:64] = block[0:64, c * 64:(c + 1) * 64]
            out[c, 64:128, 64:128] = block[64:128, c * 64:(c + 1) * 64]
        return out

    w2q = expand_w(wtab[:, 0:512])
    w2k = expand_w(wtab[:, 512:1024])
    w2v = expand_w(wtab[:, 1024:1536])
    cosH_t = httab[:, 0:32].reshape(128, 4, 8).transpose(1, 0, 2)
    sinH_t = httab[:, 32:64].reshape(128, 4, 8).transpose(1, 0, 2)
    cosM_t = httab[:, 64:96].reshape(128, 4, 8).transpose(1, 0, 2)
    sinM_t = httab[:, 96:128].reshape(128, 4, 8).transpose(1, 0, 2)
    cosV_t = httab[:, 128:384].reshape(128, 4, 64).transpose(1, 0, 2)
    sinV_t = httab[:, 384:640].reshape(128, 4, 64).transpose(1, 0, 2)
    # device-side cosL/sinL reconstruction (fp32 then bf16 round)
    cosL_t = np.zeros((4, 128, 512), dtype=np.float32)
    sinL_t = np.zeros((4, 128, 512), dtype=np.float32)
    for m in range(8):
        cm = cosM_t[:, :, m:m + 1]
        sm = sinM_t[:, :, m:m + 1]
        cosL_t[:, :, m * 64:(m + 1) * 64] = cosV_t * cm - sinV_t * sm
        sinL_t[:, :, m * 64:(m + 1) * 64] = cosV_t * sm + sinV_t * cm
    cosL_t = cosL_t.astype(BF16).astype(np.float32)
    sinL_t = sinL_t.astype(BF16).astype(np.float32)
    # reconstruct cos/sin as the device does
    cos = np.zeros((4, 128, T), dtype=np.float32)
    sin = np.zeros((4, 128, T), dtype=np.float32)
    for s in range(NS):
        ch = cosH_t[:, :, s:s + 1]
        sh = sinH_t[:, :, s:s + 1]
        cos[:, :, s * 512:(s + 1) * 512] = cosL_t * ch - sinL_t * sh
        sin[:, :, s * 512:(s + 1) * 512] = cosL_t * sh + sinL_t * ch

    def proj_T(w2):  # -> [8, 128, T]
        return np.stack([w2[c].T @ xpT[c] for c in range(8)])

    def rope(zT):
        out = np.empty_like(zT)
        for c in range(4):
            e, o = zT[c], zT[c + 4]
            out[c] = e * cos[c] - o * sin[c]
            out[c + 4] = e * sin[c] + o * cos[c]
        return out

    kT = rope(proj_T(w2k)).reshape(1024, T)
    qT = rope(proj_T(w2q)).reshape(1024, T)
    v = np.concatenate([w2v[c].T @ xpT[c] for c in range(8)], axis=0).T  # [T,1024]

    yout = np.zeros((T, 1024), dtype=np.float32)
    for G in range(NT):
        nblk = G + 1
        q = qT[:, G * 128:(G + 1) * 128].T  # [128, 1024]
        keys = kT[:, :nblk * 128]
        S = q @ keys
        k0 = 4 * ((nblk + 3) // 4 - 1) * 128
        r = G % 4
        pcol = np.arange(128)[:, None]
        ccol = np.arange(nblk * 128 - k0)[None, :]
        S[:, k0:] += np.where(ccol <= r * 128 + pcol, 0.0, -1e9)
        P = np.exp(GAMMA * S)
        yout[G * 128:(G + 1) * 128] = (P @ v[:nblk * 128]) / P.sum(1, keepdims=True)
    return yout


_NC_CACHE = {}
last_in_maps = None


def kernel(x, w_q, w_k, w_v):
    global last_in_maps
    from concourse.bass_utils import run_bass_kernel_spmd

    B, Tx, D = x.shape
    assert (B, Tx, D) == (4, 4096, 1024)
    x = np.asarray(x, dtype=np.float32)
    w2q = _block_weights(np.asarray(w_q, dtype=np.float32))
    w2k = _block_weights(np.asarray(w_k, dtype=np.float32))
    w2v = _block_weights(np.asarray(w_v, dtype=np.float32))
    tables = _rope_factor_tables()
    wtab, httab = pack_tables(w2q, w2k, w2v, tables)

    in_maps = [prep_core_inputs(x[b], wtab, httab) for b in range(4)]
    last_in_maps = in_maps

    if "nc" not in _NC_CACHE:
        _NC_CACHE["nc"] = build_nc()
    nc = _NC_CACHE["nc"]

    res = run_bass_kernel_spmd(nc, in_maps, core_ids=list(range(4)))
    out = np.zeros((B, Tx, D), dtype=np.float32)
    for b in range(4):
        out[b] = res.results[b]["y"].astype(np.float32)[:, INV_PERM]
    return out


# revision 33
# speedup vs baseline: 1.1938x; 1.1938x over previous
"""Trainium2 Bass kernel for nn_MultiHeadAttention_85229331022244.

Computation (per batch b):
  xh = x.reshape(B,T,64,16); q/k/v = per-head 64x64 projections of xh
  q,k: interleaved RoPE over the FULL 1024-dim feature axis
  scores = q @ k.T / sqrt(1024)  (single attention map over full D)
  causal softmax; y = attn @ v

Sharding: core b -> batch b (4 cores used).  One core owns the whole
batch, so x is shipped to the device exactly once and K/Q/V projections
share the same x tiles.  Host<->device traffic is the metric driver
(memory regime): RoPE cos/sin tables are reconstructed on-device from
factorized half-tables (angle addition), causal masks are generated
on-device with affine_select, and the output returns as bf16.

Device layout trick: heads are reordered even-first and paired so the
projections become 8 block-diagonal 128x128 matmuls that produce
K^T/Q^T directly in [feature-on-partition, token] layout, with RoPE
partner features living in chunk c and c+4 at the same partition index.
"""

import math
from contextlib import ExitStack

import numpy as np
import ml_dtypes

import concourse.bass as bass
import concourse.mybir as mybir
import concourse.tile as tile
from concourse import bacc
from concourse.bass import ts, ds
from concourse.masks import make_identity

BF16 = ml_dtypes.bfloat16

D_MODEL = 1024
N_HEADS = 16
HEAD_D = 64
ROPE_BASE = 10000.0
GAMMA = 1.0 / math.sqrt(D_MODEL)
T = 4096
NT = 32  # 128-row query tiles per batch
NS = 8   # 512-token stripes

# head pairs per 128-row chunk; chunks 0-3 = even heads, 4-7 = odd heads
HEAD_PAIRS = [(0, 2), (4, 6), (8, 10), (12, 14), (1, 3), (5, 7), (9, 11), (13, 15)]


def _feature_perm():
    """perm[c*128 + p] = original feature index for kernel row (c, p)."""
    perm = np.zeros(1024, dtype=np.int64)
    for c, (ha, hb) in enumerate(HEAD_PAIRS):
        for p in range(128):
            h = ha if p < 64 else hb
            perm[c * 128 + p] = (p % 64) * 16 + h
    return perm


PERM = _feature_perm()
INV_PERM = np.argsort(PERM)


def _block_weights(w):
    """w: (64, 64, 16) -> (8, 128, 128) block-diag per chunk, bf16."""
    out = np.zeros((8, 128, 128), dtype=np.float32)
    for c, (ha, hb) in enumerate(HEAD_PAIRS):
        out[c, :64, :64] = w[:, :, ha]
        out[c, 64:, 64:] = w[:, :, hb]
    return out.astype(BF16)


def _inv_freq():
    """[4, 128] rope inverse frequencies for chunks 0-3 (partners 4-7)."""
    p = np.arange(128)
    out = np.zeros((4, 128), dtype=np.float64)
    for c in range(4):
        f = (p % 64) * 8 + (2 * c + p // 64)
        out[c] = ROPE_BASE ** (-f.astype(np.float64) / 512.0)
    return out


def _rope_factor_tables():
    """Two-level angle factorization: ang(p, 512*s + 64*m + v)
    = hi(p,s) + mid(p,m) + low(p,v).  All fp32 (device rebuilds cosL/sinL
    in fp32 then rounds to bf16 once, matching single-level precision).

    Returns cosH/sinH [4,128,NS], cosM/sinM [4,128,8], cosV/sinV [4,128,64].
    """
    invf = _inv_freq()  # [4, 128]
    v = np.arange(64, dtype=np.float64)
    m = np.arange(8, dtype=np.float64) * 64.0
    s = np.arange(NS, dtype=np.float64) * 512.0
    low = invf[:, :, None] * v[None, None, :]   # [4,128,64]
    mid = invf[:, :, None] * m[None, None, :]   # [4,128,8]
    hi = invf[:, :, None] * s[None, None, :]    # [4,128,NS]
    f32 = np.float32
    return (
        np.cos(hi).astype(f32), np.sin(hi).astype(f32),
        np.cos(mid).astype(f32), np.sin(mid).astype(f32),
        np.cos(low).astype(f32), np.sin(low).astype(f32),
    )


def build_nc():
    """Build the (identical-on-all-cores) Bass program for one full batch."""
    dt = mybir.dt
    nc = bacc.Bacc("TRN2", target_bir_lowering=False)
    xpT = nc.dram_tensor("xpT", [8, 128, T], dt.bfloat16, kind="ExternalInput")
    # compact block-diag weights (only the 64 nonzero cols per row):
    # wq|wk|wv, 8 chunks x 64 cols each -> expanded on device
    wtab = nc.dram_tensor("wtab", [128, 1536], dt.bfloat16, kind="ExternalInput")
    # fp32 rope factor tables: cosH|sinH (4x8 each), cosM|sinM (4x8 each),
    # cosV|sinV (4x64 each); cosL/sinL are reconstructed on device
    httab = nc.dram_tensor("httab", [128, 640], dt.float32, kind="ExternalInput")
    y = nc.dram_tensor("y", [T, 1024], dt.bfloat16, kind="ExternalOutput")

    with tile.TileContext(nc) as tc, ExitStack() as ctx:
        const = ctx.enter_context(tc.tile_pool(name="const", bufs=1))
        kv = ctx.enter_context(tc.tile_pool(name="kv", bufs=1))
        qpool = ctx.enter_context(tc.tile_pool(name="qpool", bufs=2))
        xpool = ctx.enter_context(tc.tile_pool(name="xpool", bufs=2))
        cspool = ctx.enter_context(tc.tile_pool(name="cspool", bufs=2))
        rtmp = ctx.enter_context(tc.tile_pool(name="rtmp", bufs=2))
        ppool = ctx.enter_context(tc.tile_pool(name="ppool", bufs=2))
        ptpool = ctx.enter_context(tc.tile_pool(name="ptpool", bufs=2))
        ypool = ctx.enter_context(tc.tile_pool(name="ypool", bufs=2))
        lpool = ctx.enter_context(tc.tile_pool(name="lpool", bufs=2))
        psum = ctx.enter_context(tc.tile_pool(name="psum", bufs=2, space="PSUM"))
        # YL/YH double-buffered; V-projection PSUM shares the same slots
        # (proj and attention never need them simultaneously beyond the
        # rotation the scheduler already enforces).
        psum1 = ctx.enter_context(tc.tile_pool(name="psum1", bufs=2, space="PSUM"))

        # constants
        ident = const.tile([128, 128], dt.bfloat16, tag="ident", name="ident")
        make_identity(nc, ident)
        wtab_sb = const.tile([128, 1536], dt.bfloat16, tag="wtab", name="wtab")
        nc.sync.dma_start(wtab_sb[:], wtab[:])
        httab_sb = const.tile([128, 640], dt.float32, tag="httab", name="httab")
        nc.sync.dma_start(httab_sb[:], httab[:])
        # expand compact weights to 128x128 block-diagonal tiles
        wq_sb, wk_sb, wv_sb = [], [], []
        for wi, lst in ((0, wq_sb), (1, wk_sb), (2, wv_sb)):
            for c in range(8):
                wt = const.tile([128, 128], dt.bfloat16, tag=f"w{wi}_{c}",
                                name=f"w{wi}_{c}")
                nc.gpsimd.memset(wt[:], 0.0)
                off = wi * 512 + c * 64
                nc.vector.tensor_copy(wt[0:64, 0:64], wtab_sb[0:64, ds(off, 64)])
                nc.vector.tensor_copy(wt[64:128, 64:128],
                                      wtab_sb[64:128, ds(off, 64)])
                lst.append(wt)
        # reconstruct the rope "low" tables: cos/sin(invf * u), u = 64*m + v
        cosL_sb, sinL_sb = [], []
        for c in range(4):
            cv = httab_sb[:, ds(128 + c * 64, 64)]
            sv = httab_sb[:, ds(384 + c * 64, 64)]
            clt = const.tile([128, 512], dt.bfloat16, tag=f"cl{c}", name=f"cl{c}")
            slt = const.tile([128, 512], dt.bfloat16, tag=f"sl{c}", name=f"sl{c}")
            for m in range(NS):
                cm = httab_sb[:, ds(64 + c * NS + m, 1)]
                sm = httab_sb[:, ds(96 + c * NS + m, 1)]
                u1 = cspool.tile([128, 64], dt.float32, tag="u1", name="u1")
                u2 = cspool.tile([128, 64], dt.float32, tag="u2", name="u2")
                nc.vector.tensor_scalar_mul(u1[:], cv, cm)
                nc.vector.tensor_scalar_mul(u2[:], sv, sm)
                nc.vector.tensor_sub(clt[:, ds(m * 64, 64)], u1[:], u2[:])
                u3 = cspool.tile([128, 64], dt.float32, tag="u1", name="u3")
                u4 = cspool.tile([128, 64], dt.float32, tag="u2", name="u4")
                nc.vector.tensor_scalar_mul(u3[:], cv, sm)
                nc.vector.tensor_scalar_mul(u4[:], sv, cm)
                nc.vector.tensor_add(slt[:, ds(m * 64, 64)], u3[:], u4[:])
            cosL_sb.append(clt)
            sinL_sb.append(slt)
        cosH_sb = [httab_sb[:, ds(c * NS, NS)] for c in range(4)]
        sinH_sb = [httab_sb[:, ds(32 + c * NS, NS)] for c in range(4)]
        # causal masks for the last stripe of each q-tile: pattern depends only
        # on r = G mod 4.  mask_r[p, c] = 0 if c <= 128*r + p else -1e9.
        masks = []
        for r in range(4):
            mt = const.tile([128, 512], dt.float32, tag=f"mask{r}", name=f"mask{r}")
            nc.gpsimd.memset(mt[:], 0.0)
            nc.gpsimd.affine_select(
                out=mt[:],
                in_=mt[:],
                compare_op=mybir.AluOpType.is_ge,
                fill=-1e9,
                base=r * 128,
                pattern=[[-1, 512]],
                channel_multiplier=1,
            )
            masks.append(mt)

        # resident K^T (per chunk c and 512-key stripe s) and V (per-stripe)
        KT = {}
        for s in range(NS):
            for c in range(8):
                KT[(c, s)] = kv.tile([128, 512], dt.bfloat16, tag=f"kt{c}_{s}",
                                     name=f"kt{c}_{s}")
        V = [
            kv.tile([128, 4, 1024], dt.bfloat16, tag=f"v{s}", name=f"v{s}")
            for s in range(NS)
        ]
        QT = {}  # streamed, tags per chunk

        def emit_proj_stripe(s):
            """K^T, Q^T, V for the 512-token stripe s (shared x / cos / sin)."""
            sl = ds(s * 512, 512)
            for cp in range(4):
                xa = xpool.tile([128, 512], dt.bfloat16, tag="xa", name="xa")
                xb = xpool.tile([128, 512], dt.bfloat16, tag="xb", name="xb")
                nc.sync.dma_start(xa[:], xpT[cp, :, sl])
                nc.sync.dma_start(xb[:], xpT[cp + 4, :, sl])
                # reconstruct cos/sin for (chunk cp, stripe s) via angle addition
                cos = cspool.tile([128, 512], dt.bfloat16, tag="cos", name="cos")
                sin = cspool.tile([128, 512], dt.bfloat16, tag="sin", name="sin")
                t1 = cspool.tile([128, 512], dt.bfloat16, tag="t1", name="t1")
                t2 = cspool.tile([128, 512], dt.bfloat16, tag="t2", name="t2")
                chs = httab_sb[:, ds(cp * NS + s, 1)]
                shs = httab_sb[:, ds(32 + cp * NS + s, 1)]
                nc.vector.tensor_scalar_mul(t1[:], cosL_sb[cp][:], chs)
                nc.vector.tensor_scalar_mul(t2[:], sinL_sb[cp][:], shs)
                nc.vector.tensor_sub(cos[:], t1[:], t2[:])
                t3 = cspool.tile([128, 512], dt.bfloat16, tag="t1", name="t3")
                t4 = cspool.tile([128, 512], dt.bfloat16, tag="t2", name="t4")
                nc.vector.tensor_scalar_mul(t3[:], cosL_sb[cp][:], shs)
                nc.vector.tensor_scalar_mul(t4[:], sinL_sb[cp][:], chs)
                nc.vector.tensor_add(sin[:], t3[:], t4[:])

                def rope_pair(w_sb, out_e, out_o):
                    pe = psum.tile([128, 512], dt.float32, tag="A", name="A")
                    po = psum.tile([128, 512], dt.float32, tag="B", name="B")
                    nc.tensor.matmul(pe[:], lhsT=w_sb[cp][:], rhs=xa[:],
                                     start=True, stop=True)
                    nc.tensor.matmul(po[:], lhsT=w_sb[cp + 4][:], rhs=xb[:],
                                     start=True, stop=True)
                    ke = rtmp.tile([128, 512], dt.bfloat16, tag="ke", name="ke")
                    ko = rtmp.tile([128, 512], dt.bfloat16, tag="ko", name="ko")
                    nc.scalar.copy(ke[:], pe[:])
                    nc.scalar.copy(ko[:], po[:])
                    ta = rtmp.tile([128, 512], dt.bfloat16, tag="ta", name="ta")
                    tb = rtmp.tile([128, 512], dt.bfloat16, tag="tb", name="tb")
                    nc.vector.tensor_mul(ta[:], ke[:], cos[:])
                    nc.vector.tensor_mul(tb[:], ko[:], sin[:])
                    nc.vector.tensor_sub(out_e[:], ta[:], tb[:])
                    ta2 = rtmp.tile([128, 512], dt.bfloat16, tag="ta", name="ta")
                    tb2 = rtmp.tile([128, 512], dt.bfloat16, tag="tb", name="tb")
                    nc.vector.tensor_mul(ta2[:], ke[:], sin[:])
                    nc.vector.tensor_mul(tb2[:], ko[:], cos[:])
                    nc.vector.tensor_add(out_o[:], ta2[:], tb2[:])

                rope_pair(wk_sb, KT[(cp, s)], KT[(cp + 4, s)])
                QT[(cp, s)] = qpool.tile([128, 512], dt.bfloat16, tag=f"qt{cp}",
                                         name=f"qt{cp}")
                QT[(cp + 4, s)] = qpool.tile([128, 512], dt.bfloat16,
                                             tag=f"qt{cp + 4}", name=f"qt{cp + 4}")
                rope_pair(wq_sb, QT[(cp, s)], QT[(cp + 4, s)])

                va = psum1.tile([128, 4, 128], dt.float32, tag="YL", name="VA")
                vb = psum1.tile([128, 4, 128], dt.float32, tag="YH", name="VB")
                for sub in range(4):
                    nc.tensor.matmul(
                        va[:, sub, :], lhsT=xa[:, ts(sub, 128)], rhs=wv_sb[cp][:],
                        start=True, stop=True,
                    )
                    nc.tensor.matmul(
                        vb[:, sub, :], lhsT=xb[:, ts(sub, 128)], rhs=wv_sb[cp + 4][:],
                        start=True, stop=True,
                    )
                nc.any.tensor_copy(V[s][:, :, ds(cp * 128, 128)], va[:])
                nc.any.tensor_copy(V[s][:, :, ds((cp + 4) * 128, 128)], vb[:])

        def emit_q_tile(G):
            nblk = G + 1
            nst = (nblk + 3) // 4
            wlast = (nblk - 4 * (nst - 1)) * 128
            qs, qoff = G // 4, (G % 4) * 128
            y_lo = psum1.tile([128, 512], dt.float32, tag="YL", name="YL")
            y_hi = psum1.tile([128, 512], dt.float32, tag="YH", name="YH")
            l_parts = lpool.tile([128, NS], dt.float32, tag="lp", name="lp")
            for t in range(nst):
                w = 512 if t < nst - 1 else wlast
                S = psum.tile([128, 512], dt.float32, tag="A", name="A")
                for c in range(8):
                    nc.tensor.matmul(
                        S[:, :w],
                        lhsT=QT[(c, qs)][:, ds(qoff, 128)],
                        rhs=KT[(c, t)][:, :w],
                        start=(c == 0), stop=(c == 7),
                    )
                if t == nst - 1:
                    nc.vector.tensor_add(S[:, :w], S[:, :w], masks[G % 4][:, :w])
                P = ppool.tile([128, 512], dt.bfloat16, tag="p", name="p")
                nc.scalar.activation(
                    P[:, :w], S[:, :w], mybir.ActivationFunctionType.Exp,
                    scale=GAMMA, accum_out=l_parts[:, ds(t, 1)],
                )
                nb = w // 128
                pt_ps = psum.tile([128, 512], dt.bfloat16, tag="B", name="B")
                for b in range(nb):
                    nc.tensor.transpose(pt_ps[:, ts(b, 128)], P[:, ts(b, 128)],
                                        ident[:])
                pt = ptpool.tile([128, 512], dt.bfloat16, tag="pt", name="pt")
                nc.scalar.copy(pt[:, :w], pt_ps[:, :w])
                for b in range(nb):
                    blk = t * 4 + b
                    vs = V[blk // 4]
                    nc.tensor.matmul(y_lo[:], lhsT=pt[:, ts(b, 128)],
                                     rhs=vs[:, blk % 4, 0:512],
                                     start=(blk == 0), stop=(blk == nblk - 1))
                    nc.tensor.matmul(y_hi[:], lhsT=pt[:, ts(b, 128)],
                                     rhs=vs[:, blk % 4, 512:1024],
                                     start=(blk == 0), stop=(blk == nblk - 1))
            lsum = lpool.tile([128, 1], dt.float32, tag="ls", name="ls")
            linv = lpool.tile([128, 1], dt.float32, tag="li", name="li")
            nc.vector.tensor_reduce(lsum[:], l_parts[:, :nst],
                                    mybir.AxisListType.X, mybir.AluOpType.add)
            nc.vector.reciprocal(linv[:], lsum[:])
            y_sb = ypool.tile([128, 1024], dt.bfloat16, tag="y", name="y")
            nc.vector.tensor_scalar_mul(y_sb[:, 0:512], y_lo[:], linv[:])
            nc.vector.tensor_scalar_mul(y_sb[:, 512:1024], y_hi[:], linv[:])
            nc.sync.dma_start(y[ts(G, 128), :], y_sb[:])

        # Projection runs one stripe ahead of attention so its DMA + DVE
        # latency hides under the (PE-bound) attention of the prior stripe.
        emit_proj_stripe(0)
        for s in range(NS):
            if s + 1 < NS:
                emit_proj_stripe(s + 1)
            for G in range(4 * s, 4 * s + 4):
                emit_q_tile(G)

    nc.compile()
    return nc


# ------------------------- host side -------------------------


def _compact_w(w2):
    """[8,128,128] block-diag -> [128, 8*64] nonzero cols per row."""
    out = np.zeros((128, 8 * 64), dtype=np.float32)
    w2 = np.asarray(w2, dtype=np.float32)
    for c in range(8):
        out[0:64, c * 64:(c + 1) * 64] = w2[c, 0:64, 0:64]
        out[64:128, c * 64:(c + 1) * 64] = w2[c, 64:128, 64:128]
    return out


def pack_tables(w2q, w2k, w2v, tables):
    """wtab [128,1536] bf16 and httab [128,640] fp32 (shared by all cores)."""
    cosH_t, sinH_t, cosM_t, sinM_t, cosV_t, sinV_t = tables
    wtab = np.concatenate(
        [_compact_w(w2q), _compact_w(w2k), _compact_w(w2v)], axis=1
    ).astype(BF16)
    httab = np.concatenate(
        [
            cosH_t.transpose(1, 0, 2).reshape(128, 32),
            sinH_t.transpose(1, 0, 2).reshape(128, 32),
            cosM_t.transpose(1, 0, 2).reshape(128, 32),
            sinM_t.transpose(1, 0, 2).reshape(128, 32),
            cosV_t.transpose(1, 0, 2).reshape(128, 256),
            sinV_t.transpose(1, 0, 2).reshape(128, 256),
        ],
        axis=1,
    ).astype(np.float32)
    return wtab, httab


def prep_core_inputs(xb, wtab, httab):
    """Inputs for one core: batch slice xb (T, 1024) fp32."""
    xpT = np.ascontiguousarray(xb.T[PERM].reshape(8, 128, T)).astype(BF16)
    return {"xpT": xpT, "wtab": wtab, "httab": httab}


def core_model(inp):
    """Numpy model of one core's program (fp32 math, for tests)."""
    xpT = inp["xpT"].astype(np.float32)
    wtab = inp["wtab"].astype(np.float32)
    httab = inp["httab"].astype(np.float32)

    def expand_w(block):  # [128, 512] compact -> [8,128,128] block-diag
        out = np.zeros((8, 128, 128), dtype=np.float32)
        for c in range(8):
            out[c, 0:64, 0:64] = block[0:64, c * 64:(c + 1) * 64]
            out[c, 64:128, 64:128] = block[64:128, c * 64:(c + 1) * 64]
        return out

    w2q = expand_w(wtab[:, 0:512])
    w2k = expand_w(wtab[:, 512:1024])
    w2v = expand_w(wtab[:, 1024:1536])
    cosH_t = httab[:, 0:32].reshape(128, 4, 8).transpose(1, 0, 2)
    sinH_t = httab[:, 32:64].reshape(128, 4, 8).transpose(1, 0, 2)
    cosM_t = httab[:, 64:96].reshape(128, 4, 8).transpose(1, 0, 2)
    sinM_t = httab[:, 96:128].reshape(128, 4, 8).transpose(1, 0, 2)
    cosV_t = httab[:, 128:384].reshape(128, 4, 64).transpose(1, 0, 2)
    sinV_t = httab[:, 384:640].reshape(128, 4, 64).transpose(1, 0, 2)
    # device-side cosL/sinL reconstruction (fp32 then bf16 round)
    cosL_t = np.zeros((4, 128, 512), dtype=np.float32)
    sinL_t = np.zeros((4, 128, 512), dtype=np.float32)
    for m in range(8):
        cm = cosM_t[:, :, m:m + 1]
        sm = sinM_t[:, :, m:m + 1]
        cosL_t[:, :, m * 64:(m + 1) * 64] = cosV_t * cm - sinV_t * sm
        sinL_t[:, :, m * 64:(m + 1) * 64] = cosV_t * sm + sinV_t * cm
    cosL_t = cosL_t.astype(BF16).astype(np.float32)
    sinL_t = sinL_t.astype(BF16).astype(np.float32)
    # reconstruct cos/sin as the device does
    cos = np.zeros((4, 128, T), dtype=np.float32)
    sin = np.zeros((4, 128, T), dtype=np.float32)
    for s in range(NS):
        ch = cosH_t[:, :, s:s + 1]
        sh = sinH_t[:, :, s:s + 1]
        cos[:, :, s * 512:(s + 1) * 512] = cosL_t * ch - sinL_t * sh
        sin[:, :, s * 512:(s + 1) * 512] = cosL_t * sh + sinL_t * ch

    def proj_T(w2):  # -> [8, 128, T]
        return np.stack([w2[c].T @ xpT[c] for c in range(8)])

    def rope(zT):
        out = np.empty_like(zT)
        for c in range(4):
            e, o = zT[c], zT[c + 4]
            out[c] = e * cos[c] - o * sin[c]
            out[c + 4] = e * sin[c] + o * cos[c]
        return out

    kT = rope(proj_T(w2k)).reshape(1024, T)
    qT = rope(proj_T(w2q)).reshape(1024, T)
    v = np.concatenate([w2v[c].T @ xpT[c] for c in range(8)], axis=0).T  # [T,1024]

    yout = np.zeros((T, 1024), dtype=np.float32)
    for G in range(NT):
        nblk = G + 1
        q = qT[:, G * 128:(G + 1) * 128].T  # [128, 1024]
        keys = kT[:, :nblk * 128]
        S = q @ keys
        k0 = 4 * ((nblk + 3) // 4 - 1) * 128
        r = G % 4
        pcol = np.arange(128)[:, None]
        ccol = np.arange(nblk * 128 - k0)[None, :]
        S[:, k0:] += np.where(ccol <= r * 128 + pcol, 0.0, -1e9)
        P = np.exp(GAMMA * S)
        yout[G * 128:(G + 1) * 128] = (P @ v[:nblk * 128]) / P.sum(1, keepdims=True)
    return yout


_NC_CACHE = {}
last_in_maps = None


def kernel(x, w_q, w_k, w_v):
    global last_in_maps
    from concourse.bass_utils import run_bass_kernel_spmd

    B, Tx, D = x.shape
    assert (B, Tx, D) == (4, 4096, 1024)
    x = np.asarray(x, dtype=np.float32)
    w2q = _block_weights(np.asarray(w_q, dtype=np.float32))
    w2k = _block_weights(np.asarray(w_k, dtype=np.float32))
    w2v = _block_weights(np.asarray(w_v, dtype=np.float32))
    tables = _rope_factor_tables()
    wtab, httab = pack_tables(w2q, w2k, w2v, tables)

    in_maps = [prep_core_inputs(x[b], wtab, httab) for b in range(4)]
    last_in_maps = in_maps

    if "nc" not in _NC_CACHE:
        _NC_CACHE["nc"] = build_nc()
    nc = _NC_CACHE["nc"]

    res = run_bass_kernel_spmd(nc, in_maps, core_ids=list(range(4)))
    out = np.zeros((B, Tx, D), dtype=np.float32)
    for b in range(4):
        out[b] = res.results[b]["y"].astype(np.float32)[:, INV_PERM]
    return out


# revision 34
# speedup vs baseline: 1.2104x; 1.0138x over previous
"""Trainium2 Bass kernel for nn_MultiHeadAttention_85229331022244.

Computation (per batch b):
  xh = x.reshape(B,T,64,16); q/k/v = per-head 64x64 projections of xh
  q,k: interleaved RoPE over the FULL 1024-dim feature axis
  scores = q @ k.T / sqrt(1024)  (single attention map over full D)
  causal softmax; y = attn @ v

Sharding: core b -> batch b (4 cores used).  One core owns the whole
batch, so x is shipped to the device exactly once and K/Q/V projections
share the same x tiles.  Host<->device traffic is the metric driver
(memory regime): RoPE cos/sin tables are reconstructed on-device from
factorized half-tables (angle addition), causal masks are generated
on-device with affine_select, and the output returns as bf16.

Device layout trick: heads are reordered even-first and paired so the
projections become 8 block-diagonal 128x128 matmuls that produce
K^T/Q^T directly in [feature-on-partition, token] layout, with RoPE
partner features living in chunk c and c+4 at the same partition index.
"""

import math
from contextlib import ExitStack

import numpy as np
import ml_dtypes

import concourse.bass as bass
import concourse.mybir as mybir
import concourse.tile as tile
from concourse import bacc
from concourse.bass import ts, ds
from concourse.masks import make_identity

BF16 = ml_dtypes.bfloat16

D_MODEL = 1024
N_HEADS = 16
HEAD_D = 64
ROPE_BASE = 10000.0
GAMMA = 1.0 / math.sqrt(D_MODEL)
T = 4096
NT = 32  # 128-row query tiles per batch
NS = 8   # 512-token stripes

# head pairs per 128-row chunk; chunks 0-3 = even heads, 4-7 = odd heads
HEAD_PAIRS = [(0, 2), (4, 6), (8, 10), (12, 14), (1, 3), (5, 7), (9, 11), (13, 15)]


def _feature_perm():
    """perm[c*128 + p] = original feature index for kernel row (c, p)."""
    perm = np.zeros(1024, dtype=np.int64)
    for c, (ha, hb) in enumerate(HEAD_PAIRS):
        for p in range(128):
            h = ha if p < 64 else hb
            perm[c * 128 + p] = (p % 64) * 16 + h
    return perm


PERM = _feature_perm()
INV_PERM = np.argsort(PERM)


def _block_weights(w):
    """w: (64, 64, 16) -> (8, 128, 128) block-diag per chunk, bf16."""
    out = np.zeros((8, 128, 128), dtype=np.float32)
    for c, (ha, hb) in enumerate(HEAD_PAIRS):
        out[c, :64, :64] = w[:, :, ha]
        out[c, 64:, 64:] = w[:, :, hb]
    return out.astype(BF16)


def _inv_freq():
    """[4, 128] rope inverse frequencies for chunks 0-3 (partners 4-7)."""
    p = np.arange(128)
    out = np.zeros((4, 128), dtype=np.float64)
    for c in range(4):
        f = (p % 64) * 8 + (2 * c + p // 64)
        out[c] = ROPE_BASE ** (-f.astype(np.float64) / 512.0)
    return out


def _rope_factor_tables():
    """Two-level angle factorization: ang(p, 512*s + 64*m + v)
    = hi(p,s) + mid(p,m) + low(p,v).  All fp32 (device rebuilds cosL/sinL
    in fp32 then rounds to bf16 once, matching single-level precision).

    Returns cosH/sinH [4,128,NS], cosM/sinM [4,128,8], cosV/sinV [4,128,64].
    """
    invf = _inv_freq()  # [4, 128]
    v = np.arange(64, dtype=np.float64)
    m = np.arange(8, dtype=np.float64) * 64.0
    s = np.arange(NS, dtype=np.float64) * 512.0
    low = invf[:, :, None] * v[None, None, :]   # [4,128,64]
    mid = invf[:, :, None] * m[None, None, :]   # [4,128,8]
    hi = invf[:, :, None] * s[None, None, :]    # [4,128,NS]
    f32 = np.float32
    return (
        np.cos(hi).astype(f32), np.sin(hi).astype(f32),
        np.cos(mid).astype(f32), np.sin(mid).astype(f32),
        np.cos(low).astype(f32), np.sin(low).astype(f32),
    )


def build_nc():
    """Build the (identical-on-all-cores) Bass program for one full batch."""
    dt = mybir.dt
    nc = bacc.Bacc("TRN2", target_bir_lowering=False)
    xpT = nc.dram_tensor("xpT", [8, 128, T], dt.bfloat16, kind="ExternalInput")
    # compact block-diag weights (only the 64 nonzero cols per row):
    # wq|wk|wv, 8 chunks x 64 cols each -> expanded on device
    wtab = nc.dram_tensor("wtab", [128, 1536], dt.bfloat16, kind="ExternalInput")
    # fp32 rope factor tables: cosH|sinH (4x8 each), cosM|sinM (4x8 each),
    # cosV|sinV (4x64 each); cosL/sinL are reconstructed on device
    httab = nc.dram_tensor("httab", [128, 640], dt.float32, kind="ExternalInput")
    y = nc.dram_tensor("y", [T, 1024], dt.bfloat16, kind="ExternalOutput")

    with tile.TileContext(nc) as tc, ExitStack() as ctx:
        const = ctx.enter_context(tc.tile_pool(name="const", bufs=1))
        kv = ctx.enter_context(tc.tile_pool(name="kv", bufs=1))
        qpool = ctx.enter_context(tc.tile_pool(name="qpool", bufs=2))
        xpool = ctx.enter_context(tc.tile_pool(name="xpool", bufs=2))
        cspool = ctx.enter_context(tc.tile_pool(name="cspool", bufs=2))
        rtmp = ctx.enter_context(tc.tile_pool(name="rtmp", bufs=2))
        ppool = ctx.enter_context(tc.tile_pool(name="ppool", bufs=2))
        ptpool = ctx.enter_context(tc.tile_pool(name="ptpool", bufs=2))
        ypool = ctx.enter_context(tc.tile_pool(name="ypool", bufs=2))
        lpool = ctx.enter_context(tc.tile_pool(name="lpool", bufs=2))
        psum = ctx.enter_context(tc.tile_pool(name="psum", bufs=2, space="PSUM"))
        # YL/YH double-buffered; V-projection PSUM shares the same slots
        # (proj and attention never need them simultaneously beyond the
        # rotation the scheduler already enforces).
        psum1 = ctx.enter_context(tc.tile_pool(name="psum1", bufs=2, space="PSUM"))

        # constants
        ident = const.tile([128, 128], dt.bfloat16, tag="ident", name="ident")
        make_identity(nc, ident)
        wtab_sb = const.tile([128, 1536], dt.bfloat16, tag="wtab", name="wtab")
        nc.sync.dma_start(wtab_sb[:], wtab[:])
        httab_sb = const.tile([128, 640], dt.float32, tag="httab", name="httab")
        nc.sync.dma_start(httab_sb[:], httab[:])
        # expand compact weights (ScalarE copies — keeps DVE free for rope
        # reconstruction) and rebuild the rope "low" tables, interleaved per
        # chunk in first-consumption order so chunk 0 is ready ASAP.
        wq_sb, wk_sb, wv_sb = [None] * 8, [None] * 8, [None] * 8
        cosL_sb, sinL_sb = [], []

        def expand_weight(wi, lst, c):
            wt = const.tile([128, 128], dt.bfloat16, tag=f"w{wi}_{c}",
                            name=f"w{wi}_{c}")
            nc.gpsimd.memset(wt[:], 0.0)
            off = wi * 512 + c * 64
            nc.scalar.copy(wt[0:64, 0:64], wtab_sb[0:64, ds(off, 64)])
            nc.scalar.copy(wt[64:128, 64:128], wtab_sb[64:128, ds(off, 64)])
            lst[c] = wt

        for cp in range(4):
            # rope low tables for chunk cp (shared with partner cp+4)
            cv = httab_sb[:, ds(128 + cp * 64, 64)]
            sv = httab_sb[:, ds(384 + cp * 64, 64)]
            clt = const.tile([128, 512], dt.bfloat16, tag=f"cl{cp}", name=f"cl{cp}")
            slt = const.tile([128, 512], dt.bfloat16, tag=f"sl{cp}", name=f"sl{cp}")
            for m in range(NS):
                cm = httab_sb[:, ds(64 + cp * NS + m, 1)]
                sm = httab_sb[:, ds(96 + cp * NS + m, 1)]
                u1 = cspool.tile([128, 64], dt.float32, tag="u1", name="u1")
                u2 = cspool.tile([128, 64], dt.float32, tag="u2", name="u2")
                nc.vector.tensor_scalar_mul(u1[:], cv, cm)
                nc.vector.tensor_scalar_mul(u2[:], sv, sm)
                nc.vector.tensor_sub(clt[:, ds(m * 64, 64)], u1[:], u2[:])
                u3 = cspool.tile([128, 64], dt.float32, tag="u1", name="u3")
                u4 = cspool.tile([128, 64], dt.float32, tag="u2", name="u4")
                nc.vector.tensor_scalar_mul(u3[:], cv, sm)
                nc.vector.tensor_scalar_mul(u4[:], sv, cm)
                nc.vector.tensor_add(slt[:, ds(m * 64, 64)], u3[:], u4[:])
            cosL_sb.append(clt)
            sinL_sb.append(slt)
            for wi, lst in ((1, wk_sb), (0, wq_sb), (2, wv_sb)):
                expand_weight(wi, lst, cp)
                expand_weight(wi, lst, cp + 4)
        cosH_sb = [httab_sb[:, ds(c * NS, NS)] for c in range(4)]
        sinH_sb = [httab_sb[:, ds(32 + c * NS, NS)] for c in range(4)]
        # causal masks for the last stripe of each q-tile: pattern depends only
        # on r = G mod 4.  mask_r[p, c] = 0 if c <= 128*r + p else -1e9.
        masks = []
        for r in range(4):
            mt = const.tile([128, 512], dt.float32, tag=f"mask{r}", name=f"mask{r}")
            nc.gpsimd.memset(mt[:], 0.0)
            nc.gpsimd.affine_select(
                out=mt[:],
                in_=mt[:],
                compare_op=mybir.AluOpType.is_ge,
                fill=-1e9,
                base=r * 128,
                pattern=[[-1, 512]],
                channel_multiplier=1,
            )
            masks.append(mt)

        # resident K^T (per chunk c and 512-key stripe s) and V (per-stripe)
        KT = {}
        for s in range(NS):
            for c in range(8):
                KT[(c, s)] = kv.tile([128, 512], dt.bfloat16, tag=f"kt{c}_{s}",
                                     name=f"kt{c}_{s}")
        V = [
            kv.tile([128, 4, 1024], dt.bfloat16, tag=f"v{s}", name=f"v{s}")
            for s in range(NS)
        ]
        QT = {}  # streamed, tags per chunk

        def emit_proj_stripe(s):
            """K^T, Q^T, V for the 512-token stripe s (shared x / cos / sin)."""
            sl = ds(s * 512, 512)
            for cp in range(4):
                xa = xpool.tile([128, 512], dt.bfloat16, tag="xa", name="xa")
                xb = xpool.tile([128, 512], dt.bfloat16, tag="xb", name="xb")
                nc.sync.dma_start(xa[:], xpT[cp, :, sl])
                nc.sync.dma_start(xb[:], xpT[cp + 4, :, sl])
                # reconstruct cos/sin for (chunk cp, stripe s) via angle addition
                cos = cspool.tile([128, 512], dt.bfloat16, tag="cos", name="cos")
                sin = cspool.tile([128, 512], dt.bfloat16, tag="sin", name="sin")
                t1 = cspool.tile([128, 512], dt.bfloat16, tag="t1", name="t1")
                t2 = cspool.tile([128, 512], dt.bfloat16, tag="t2", name="t2")
                chs = httab_sb[:, ds(cp * NS + s, 1)]
                shs = httab_sb[:, ds(32 + cp * NS + s, 1)]
                nc.vector.tensor_scalar_mul(t1[:], cosL_sb[cp][:], chs)
                nc.vector.tensor_scalar_mul(t2[:], sinL_sb[cp][:], shs)
                nc.vector.tensor_sub(cos[:], t1[:], t2[:])
                t3 = cspool.tile([128, 512], dt.bfloat16, tag="t1", name="t3")
                t4 = cspool.tile([128, 512], dt.bfloat16, tag="t2", name="t4")
                nc.vector.tensor_scalar_mul(t3[:], cosL_sb[cp][:], shs)
                nc.vector.tensor_scalar_mul(t4[:], sinL_sb[cp][:], chs)
                nc.vector.tensor_add(sin[:], t3[:], t4[:])

                def rope_pair(w_sb, out_e, out_o):
                    pe = psum.tile([128, 512], dt.float32, tag="A", name="A")
                    po = psum.tile([128, 512], dt.float32, tag="B", name="B")
                    nc.tensor.matmul(pe[:], lhsT=w_sb[cp][:], rhs=xa[:],
                                     start=True, stop=True)
                    nc.tensor.matmul(po[:], lhsT=w_sb[cp + 4][:], rhs=xb[:],
                                     start=True, stop=True)
                    ke = rtmp.tile([128, 512], dt.bfloat16, tag="ke", name="ke")
                    ko = rtmp.tile([128, 512], dt.bfloat16, tag="ko", name="ko")
                    nc.scalar.copy(ke[:], pe[:])
                    nc.scalar.copy(ko[:], po[:])
                    ta = rtmp.tile([128, 512], dt.bfloat16, tag="ta", name="ta")
                    tb = rtmp.tile([128, 512], dt.bfloat16, tag="tb", name="tb")
                    nc.vector.tensor_mul(ta[:], ke[:], cos[:])
                    nc.vector.tensor_mul(tb[:], ko[:], sin[:])
                    nc.vector.tensor_sub(out_e[:], ta[:], tb[:])
                    ta2 = rtmp.tile([128, 512], dt.bfloat16, tag="ta", name="ta")
                    tb2 = rtmp.tile([128, 512], dt.bfloat16, tag="tb", name="tb")
                    nc.vector.tensor_mul(ta2[:], ke[:], sin[:])
                    nc.vector.tensor_mul(tb2[:], ko[:], cos[:])
                    nc.vector.tensor_add(out_o[:], ta2[:], tb2[:])

                rope_pair(wk_sb, KT[(cp, s)], KT[(cp + 4, s)])
                QT[(cp, s)] = qpool.tile([128, 512], dt.bfloat16, tag=f"qt{cp}",
                                         name=f"qt{cp}")
                QT[(cp + 4, s)] = qpool.tile([128, 512], dt.bfloat16,
                                             tag=f"qt{cp + 4}", name=f"qt{cp + 4}")
                rope_pair(wq_sb, QT[(cp, s)], QT[(cp + 4, s)])

                va = psum1.tile([128, 4, 128], dt.float32, tag="YL", name="VA")
                vb = psum1.tile([128, 4, 128], dt.float32, tag="YH", name="VB")
                for sub in range(4):
                    nc.tensor.matmul(
                        va[:, sub, :], lhsT=xa[:, ts(sub, 128)], rhs=wv_sb[cp][:],
                        start=True, stop=True,
                    )
                    nc.tensor.matmul(
                        vb[:, sub, :], lhsT=xb[:, ts(sub, 128)], rhs=wv_sb[cp + 4][:],
                        start=True, stop=True,
                    )
                nc.any.tensor_copy(V[s][:, :, ds(cp * 128, 128)], va[:])
                nc.any.tensor_copy(V[s][:, :, ds((cp + 4) * 128, 128)], vb[:])

        def emit_q_tile(G):
            nblk = G + 1
            nst = (nblk + 3) // 4
            wlast = (nblk - 4 * (nst - 1)) * 128
            qs, qoff = G // 4, (G % 4) * 128
            y_lo = psum1.tile([128, 512], dt.float32, tag="YL", name="YL")
            y_hi = psum1.tile([128, 512], dt.float32, tag="YH", name="YH")
            l_parts = lpool.tile([128, NS], dt.float32, tag="lp", name="lp")
            for t in range(nst):
                w = 512 if t < nst - 1 else wlast
                S = psum.tile([128, 512], dt.float32, tag="A", name="A")
                for c in range(8):
                    nc.tensor.matmul(
                        S[:, :w],
                        lhsT=QT[(c, qs)][:, ds(qoff, 128)],
                        rhs=KT[(c, t)][:, :w],
                        start=(c == 0), stop=(c == 7),
                    )
                if t == nst - 1:
                    nc.vector.tensor_add(S[:, :w], S[:, :w], masks[G % 4][:, :w])
                P = ppool.tile([128, 512], dt.bfloat16, tag="p", name="p")
                nc.scalar.activation(
                    P[:, :w], S[:, :w], mybir.ActivationFunctionType.Exp,
                    scale=GAMMA, accum_out=l_parts[:, ds(t, 1)],
                )
                nb = w // 128
                pt_ps = psum.tile([128, 512], dt.bfloat16, tag="B", name="B")
                for b in range(nb):
                    nc.tensor.transpose(pt_ps[:, ts(b, 128)], P[:, ts(b, 128)],
                                        ident[:])
                pt = ptpool.tile([128, 512], dt.bfloat16, tag="pt", name="pt")
                nc.scalar.copy(pt[:, :w], pt_ps[:, :w])
                for b in range(nb):
                    blk = t * 4 + b
                    vs = V[blk // 4]
                    nc.tensor.matmul(y_lo[:], lhsT=pt[:, ts(b, 128)],
                                     rhs=vs[:, blk % 4, 0:512],
                                     start=(blk == 0), stop=(blk == nblk - 1))
                    nc.tensor.matmul(y_hi[:], lhsT=pt[:, ts(b, 128)],
                                     rhs=vs[:, blk % 4, 512:1024],
                                     start=(blk == 0), stop=(blk == nblk - 1))
            lsum = lpool.tile([128, 1], dt.float32, tag="ls", name="ls")
            linv = lpool.tile([128, 1], dt.float32, tag="li", name="li")
            nc.vector.tensor_reduce(lsum[:], l_parts[:, :nst],
                                    mybir.AxisListType.X, mybir.AluOpType.add)
            nc.vector.reciprocal(linv[:], lsum[:])
            y_sb = ypool.tile([128, 1024], dt.bfloat16, tag="y", name="y")
            nc.vector.tensor_scalar_mul(y_sb[:, 0:512], y_lo[:], linv[:])
            nc.vector.tensor_scalar_mul(y_sb[:, 512:1024], y_hi[:], linv[:])
            nc.sync.dma_start(y[ts(G, 128), :], y_sb[:])

        # Projection runs one stripe ahead of attention so its DMA + DVE
        # latency hides under the (PE-bound) attention of the prior stripe.
        emit_proj_stripe(0)
        for s in range(NS):
            if s + 1 < NS:
                emit_proj_stripe(s + 1)
            for G in range(4 * s, 4 * s + 4):
                emit_q_tile(G)

    nc.compile()
    return nc


# ------------------------- host side -------------------------


def _compact_w(w2):
    """[8,128,128] block-diag -> [128, 8*64] nonzero cols per row."""
    out = np.zeros((128, 8 * 64), dtype=np.float32)
    w2 = np.asarray(w2, dtype=np.float32)
    for c in range(8):
        out[0:64, c * 64:(c + 1) * 64] = w2[c, 0:64, 0:64]
        out[64:128, c * 64:(c + 1) * 64] = w2[c, 64:128, 64:128]
    return out


def pack_tables(w2q, w2k, w2v, tables):
    """wtab [128,1536] bf16 and httab [128,640] fp32 (shared by all cores)."""
    cosH_t, sinH_t, cosM_t, sinM_t, cosV_t, sinV_t = tables
    wtab = np.concatenate(
        [_compact_w(w2q), _compact_w(w2k), _compact_w(w2v)], axis=1
    ).astype(BF16)
    httab = np.concatenate(
        [
            cosH_t.transpose(1, 0, 2).reshape(128, 32),
            sinH_t.transpose(1, 0, 2).reshape(128, 32),
            cosM_t.transpose(1, 0, 2).reshape(128, 32),
            sinM_t.transpose(1, 0, 2).reshape(128, 32),
            cosV_t.transpose(1, 0, 2).reshape(128, 256),
            sinV_t.transpose(1, 0, 2).reshape(128, 256),
        ],
        axis=1,
    ).astype(np.float32)
    return wtab, httab


def prep_core_inputs(xb, wtab, httab):
    """Inputs for one core: batch slice xb (T, 1024) fp32."""
    xpT = np.ascontiguousarray(xb.T[PERM].reshape(8, 128, T)).astype(BF16)
    return {"xpT": xpT, "wtab": wtab, "httab": httab}


def core_model(inp):
    """Numpy model of one core's program (fp32 math, for tests)."""
    xpT = inp["xpT"].astype(np.float32)
    wtab = inp["wtab"].astype(np.float32)
    httab = inp["httab"].astype(np.float32)

    def expand_w(block):  # [128, 512] compact -> [8,128,128] block-diag
        out = np.zeros((8, 128, 128), dtype=np.float32)
        for c in range(8):
            out[c, 0:64, 0:64] = block[0:64, c * 64:(c + 1) * 64]
            out[c, 64:128, 64:128] = block[64:128, c * 64:(c + 1) * 64]
        return out

    w2q = expand_w(wtab[:, 0:512])
    w2k = expand_w(wtab[:, 512:1024])
    w2v = expand_w(wtab[:, 1024:1536])
    cosH_t = httab[:, 0:32].reshape(128, 4, 8).transpose(1, 0, 2)
    sinH_t = httab[:, 32:64].reshape(128, 4, 8).transpose(1, 0, 2)
    cosM_t = httab[:, 64:96].reshape(128, 4, 8).transpose(1, 0, 2)
    sinM_t = httab[:, 96:128].reshape(128, 4, 8).transpose(1, 0, 2)
    cosV_t = httab[:, 128:384].reshape(128, 4, 64).transpose(1, 0, 2)
    sinV_t = httab[:, 384:640].reshape(128, 4, 64).transpose(1, 0, 2)
    # device-side cosL/sinL reconstruction (fp32 then bf16 round)
    cosL_t = np.zeros((4, 128, 512), dtype=np.float32)
    sinL_t = np.zeros((4, 128, 512), dtype=np.float32)
    for m in range(8):
        cm = cosM_t[:, :, m:m + 1]
        sm = sinM_t[:, :, m:m + 1]
        cosL_t[:, :, m * 64:(m + 1) * 64] = cosV_t * cm - sinV_t * sm
        sinL_t[:, :, m * 64:(m + 1) * 64] = cosV_t * sm + sinV_t * cm
    cosL_t = cosL_t.astype(BF16).astype(np.float32)
    sinL_t = sinL_t.astype(BF16).astype(np.float32)
    # reconstruct cos/sin as the device does
    cos = np.zeros((4, 128, T), dtype=np.float32)
    sin = np.zeros((4, 128, T), dtype=np.float32)
    for s in range(NS):
        ch = cosH_t[:, :, s:s + 1]
        sh = sinH_t[:, :, s:s + 1]
        cos[:, :, s * 512:(s + 1) * 512] = cosL_t * ch - sinL_t * sh
        sin[:, :, s * 512:(s + 1) * 512] = cosL_t * sh + sinL_t * ch

    def proj_T(w2):  # -> [8, 128, T]
        return np.stack([w2[c].T @ xpT[c] for c in range(8)])

    def rope(zT):
        out = np.empty_like(zT)
        for c in range(4):
            e, o = zT[c], zT[c + 4]
            out[c] = e * cos[c] - o * sin[c]
            out[c + 4] = e * sin[c] + o * cos[c]
        return out

    kT = rope(proj_T(w2k)).reshape(1024, T)
    qT = rope(proj_T(w2q)).reshape(1024, T)
    v = np.concatenate([w2v[c].T @ xpT[c] for c in range(8)], axis=0).T  # [T,1024]

    yout = np.zeros((T, 1024), dtype=np.float32)
    for G in range(NT):
        nblk = G + 1
        q = qT[:, G * 128:(G + 1) * 128].T  # [128, 1024]
        keys = kT[:, :nblk * 128]
        S = q @ keys
        k0 = 4 * ((nblk + 3) // 4 - 1) * 128
        r = G % 4
        pcol = np.arange(128)[:, None]
        ccol = np.arange(nblk * 128 - k0)[None, :]
        S[:, k0:] += np.where(ccol <= r * 128 + pcol, 0.0, -1e9)
        P = np.exp(GAMMA * S)
        yout[G * 128:(G + 1) * 128] = (P @ v[:nblk * 128]) / P.sum(1, keepdims=True)
    return yout


_NC_CACHE = {}
last_in_maps = None


def kernel(x, w_q, w_k, w_v):
    global last_in_maps
    from concourse.bass_utils import run_bass_kernel_spmd

    B, Tx, D = x.shape
    assert (B, Tx, D) == (4, 4096, 1024)
    x = np.asarray(x, dtype=np.float32)
    w2q = _block_weights(np.asarray(w_q, dtype=np.float32))
    w2k = _block_weights(np.asarray(w_k, dtype=np.float32))
    w2v = _block_weights(np.asarray(w_v, dtype=np.float32))
    tables = _rope_factor_tables()
    wtab, httab = pack_tables(w2q, w2k, w2v, tables)

    in_maps = [prep_core_inputs(x[b], wtab, httab) for b in range(4)]
    last_in_maps = in_maps

    if "nc" not in _NC_CACHE:
        _NC_CACHE["nc"] = build_nc()
    nc = _NC_CACHE["nc"]

    res = run_bass_kernel_spmd(nc, in_maps, core_ids=list(range(4)))
    out = np.zeros((B, Tx, D), dtype=np.float32)
    for b in range(4):
        out[b] = res.results[b]["y"].astype(np.float32)[:, INV_PERM]
    return out
